# revision 7
# baseline (speedup 1.0000x reference)
"""Trainium2 Bass kernel for the e3nn-style InterModule:
   out = Linear2( NormAct( Linear1(x) ) )  over irreps
     IN  [(512,0),(256,1),(128,2)]  dim 1920
     MID [(1024,0),(512,1),(256,2)] dim 3840
     OUT = IN

Strategy (per core, data-parallel over N across 8 cores):
  - node blocks of 512; x loaded node-major, PE-transposed into a
    j-de-interleaved feature-major layout X^T (f32r)
  - Linear1: stationary W1 (f32r), moving X^T -> H^T in PSUM (fp32)
  - NormAct: nsq/sqrt on the sqrt ACT-set, sigmoid on the sigmoid set
    (2 table loads per block), g = h * sigmoid(||h||) stored f32r
  - Linear2: stationary G^T chunks, moving W2 -> node-major out in PSUM,
    interleave-assembled in SBUF, DMA'd out
  - All matmuls float32r (TF32-class, 1 cyc/row at free>=256)

Host side: shard x over 8 cores, prescale weights by 1/sqrt(mul_in).
"""

import math
from contextlib import ExitStack

import numpy as np

import concourse.bass as bass
import concourse.tile as tile
from concourse import bacc, mybir
from concourse.bass_utils import run_bass_kernel_spmd
from concourse.masks import make_identity

F32 = mybir.dt.float32
F32R = mybir.dt.float32r
AF = mybir.ActivationFunctionType
ALU = mybir.AluOpType

N_CORES = 8
N_TOTAL = 32768
N_CORE = N_TOTAL // N_CORES          # 4096
BLK = 512                            # nodes per block
NBLK = N_CORE // BLK                 # 8
NSUB = BLK // 128                    # 4

D_IN = 1920
D_OUT = 1920

# feature-tile order for X^T: l0 k0..3 | l1 (k,j) | l2 j0..4
FT_L0 = 0      # + k               (4 tiles)
FT_L1 = 4      # + k*3 + j         (6 tiles)
FT_L2 = 10     # + j               (5 tiles)
N_FT = 15


def _build():
    nc = bacc.Bacc(
        "TRN2", target_bir_lowering=False, debug=False, num_devices=N_CORES
    )

    x = nc.dram_tensor("x", [N_CORE, D_IN], F32, kind="ExternalInput").ap()
    w1_l0 = nc.dram_tensor("w1_l0", [512, 1024], F32, kind="ExternalInput").ap()
    w1_l1 = nc.dram_tensor("w1_l1", [256, 512], F32, kind="ExternalInput").ap()
    w1_l2 = nc.dram_tensor("w1_l2", [128, 256], F32, kind="ExternalInput").ap()
    w2_l0 = nc.dram_tensor("w2_l0", [1024, 512], F32, kind="ExternalInput").ap()
    w2_l1 = nc.dram_tensor("w2_l1", [512, 256], F32, kind="ExternalInput").ap()
    w2_l2 = nc.dram_tensor("w2_l2", [256, 128], F32, kind="ExternalInput").ap()
    out = nc.dram_tensor("out", [N_CORE, D_OUT], F32, kind="ExternalOutput").ap()

    with tile.TileContext(nc) as tc, ExitStack() as ctx:
        consts = ctx.enter_context(tc.tile_pool(name="consts", bufs=1))
        sb = ctx.enter_context(tc.tile_pool(name="sb", bufs=1))
        ps = ctx.enter_context(tc.tile_pool(name="ps", bufs=1, space="PSUM"))

        ident = consts.tile([128, 128], F32)
        make_identity(nc, ident)

        # ---- weights: DMA straight into f32r tiles (bitcast, PE rounds) ----
        w1r_l0 = consts.tile([128, 4, 1024], F32R)
        w1r_l1 = consts.tile([128, 2, 512], F32R)
        w1r_l2 = consts.tile([128, 256], F32R)
        w2r_l0 = consts.tile([128, 8, 512], F32R)
        w2r_l1 = consts.tile([128, 4, 256], F32R)
        w2r_l2 = consts.tile([128, 2, 128], F32R)
        nc.sync.dma_start(
            out=w1r_l0,
            in_=w1_l0.bitcast(F32R).rearrange("(t p) v -> p t v", p=128),
        )
        nc.sync.dma_start(
            out=w1r_l1,
            in_=w1_l1.bitcast(F32R).rearrange("(t p) v -> p t v", p=128),
        )
        nc.sync.dma_start(out=w1r_l2, in_=w1_l2.bitcast(F32R))
        nc.sync.dma_start(
            out=w2r_l0,
            in_=w2_l0.bitcast(F32R).rearrange("(t p) v -> p t v", p=128),
        )
        nc.sync.dma_start(
            out=w2r_l1,
            in_=w2_l1.bitcast(F32R).rearrange("(t p) v -> p t v", p=128),
        )
        nc.sync.dma_start(
            out=w2r_l2,
            in_=w2_l2.bitcast(F32R).rearrange("(t p) v -> p t v", p=128),
        )

        # ---- persistent per-block SBUF ----
        xt = sb.tile([128, N_FT, BLK], F32R, name="xt")        # X^T, de-interleaved
        g_l0 = sb.tile([128, 8, BLK], F32R, name="g_l0")
        g_l1 = sb.tile([128, 4, 3, BLK], F32R, name="g_l1")
        g_l2 = sb.tile([128, 2, 5, BLK], F32R, name="g_l2")
        nbuf = sb.tile([128, 6, BLK], F32, name="nbuf")        # norms (l1 x4, l2 x2)

        for b in range(NBLK):
            # ================= stage 1: load + transpose =================
            x_nat = sb.tile([128, NSUB, D_IN], F32, name="x_nat", tag="x_nat", bufs=1)
            nc.sync.dma_start(
                out=x_nat,
                in_=x[b * BLK : (b + 1) * BLK, :].rearrange(
                    "(s p) f -> p s f", p=128
                ),
            )
            def tr_src(ft, s):
                if ft < FT_L1:
                    k = ft
                    return x_nat[:, s, k * 128 : (k + 1) * 128]
                if ft < FT_L2:
                    k, j = divmod(ft - FT_L1, 3)
                    v = x_nat[:, s, 512:1280].rearrange("p (u j) -> p u j", j=3)
                    return v[:, k * 128 : (k + 1) * 128, j]
                j = ft - FT_L2
                v = x_nat[:, s, 1280:1920].rearrange("p (u j) -> p u j", j=5)
                return v[:, :, j]

            for grp in range(5):  # 5 groups x 3 feature tiles
                st = ps.tile([128, 3, BLK], F32, name="st", tag="hps", bufs=2)
                for c in range(3):
                    ft = grp * 3 + c
                    for s in range(NSUB):
                        nc.tensor.transpose(
                            st[:, c, s * 128 : (s + 1) * 128], tr_src(ft, s), ident
                        )
                nc.vector.tensor_copy(
                    out=xt[:, grp * 3 : (grp + 1) * 3, :], in_=st
                )

            # ============ phase A (sqrt set): l1 + l2 of Linear1 ============
            # l1: mid muls 512 -> kv 0..3, contraction 256 -> ki 0..1
            for kv in range(4):
                hm = ps.tile([128, 3, BLK], F32, name="hm", tag="hps", bufs=2)
                for j in range(3):
                    for ki in range(2):
                        nc.tensor.matmul(
                            hm[:, j, :],
                            w1r_l1[:, ki, kv * 128 : (kv + 1) * 128],
                            xt[:, FT_L1 + ki * 3 + j, :],
                            start=(ki == 0),
                            stop=(ki == 1),
                        )
                nc.scalar.activation(out=g_l1[:, kv], in_=hm, func=AF.Copy)
                sq = sb.tile([128, 3, BLK], F32, name="sq", tag="sq", bufs=2)
                nc.vector.tensor_mul(sq, hm, g_l1[:, kv])
                nsq = nbuf[:, kv, :]
                nc.vector.tensor_add(nsq, sq[:, 0, :], sq[:, 1, :])
                nc.vector.tensor_add(nsq, nsq, sq[:, 2, :])
                nc.scalar.activation(out=nsq, in_=nsq, func=AF.Sqrt)
            # l2: mid muls 256 -> kv 0..1, contraction 128 (single ki), 5 j's
            for kv in range(2):
                hm1 = ps.tile([128, 3, BLK], F32, name="hm1", tag="hps", bufs=2)
                for j in range(3):
                    nc.tensor.matmul(
                        hm1[:, j, :],
                        w1r_l2[:, kv * 128 : (kv + 1) * 128],
                        xt[:, FT_L2 + j, :],
                        start=True,
                        stop=True,
                    )
                hm2 = ps.tile([128, 3, BLK], F32, name="hm2", tag="hps", bufs=2)
                for j in range(3, 5):
                    nc.tensor.matmul(
                        hm2[:, j - 3, :],
                        w1r_l2[:, kv * 128 : (kv + 1) * 128],
                        xt[:, FT_L2 + j, :],
                        start=True,
                        stop=True,
                    )
                nc.scalar.activation(out=g_l2[:, kv, 0:3, :], in_=hm1, func=AF.Copy)
                nc.scalar.activation(
                    out=g_l2[:, kv, 3:5, :], in_=hm2[:, 0:2, :], func=AF.Copy
                )
                sq1 = sb.tile([128, 3, BLK], F32, name="sq1", tag="sq", bufs=2)
                nc.vector.tensor_mul(sq1, hm1, g_l2[:, kv, 0:3, :])
                sq2 = sb.tile([128, 3, BLK], F32, name="sq2", tag="sq", bufs=2)
                nc.vector.tensor_mul(
                    sq2[:, 0:2, :], hm2[:, 0:2, :], g_l2[:, kv, 3:5, :]
                )
                nsq = nbuf[:, 4 + kv, :]
                nc.vector.tensor_add(nsq, sq1[:, 0, :], sq1[:, 1, :])
                nc.vector.tensor_add(nsq, nsq, sq1[:, 2, :])
                nc.vector.tensor_add(nsq, nsq, sq2[:, 0, :])
                nc.vector.tensor_add(nsq, nsq, sq2[:, 1, :])
                nc.scalar.activation(out=nsq, in_=nsq, func=AF.Sqrt)

            # ========= phase B (sigmoid set): l0 of Linear1 + scales =========
            for kv in range(8):
                h1 = ps.tile([128, BLK], F32, name="h1", tag="ps1", bufs=2)
                for ki in range(4):
                    nc.tensor.matmul(
                        h1,
                        w1r_l0[:, ki, kv * 128 : (kv + 1) * 128],
                        xt[:, FT_L0 + ki, :],
                        start=(ki == 0),
                        stop=(ki == 3),
                    )
                n0 = sb.tile([128, BLK], F32, name="n0", tag="s", bufs=3)
                nc.scalar.activation(out=n0, in_=h1, func=AF.Abs)
                nc.scalar.activation(out=n0, in_=n0, func=AF.Sigmoid)
                nc.vector.tensor_mul(g_l0[:, kv, :], h1, n0)
            for kv in range(4):
                sl = sb.tile([128, BLK], F32, name="sl", tag="s", bufs=3)
                nc.scalar.activation(out=sl, in_=nbuf[:, kv, :], func=AF.Sigmoid)
                nc.vector.tensor_mul(
                    g_l1[:, kv],
                    g_l1[:, kv],
                    sl.unsqueeze(1).broadcast_to([128, 3, BLK]),
                )
            for kv in range(2):
                sl2 = sb.tile([128, BLK], F32, name="sl2", tag="s", bufs=3)
                nc.scalar.activation(out=sl2, in_=nbuf[:, 4 + kv, :], func=AF.Sigmoid)
                nc.vector.tensor_mul(
                    g_l2[:, kv],
                    g_l2[:, kv],
                    sl2.unsqueeze(1).broadcast_to([128, 5, BLK]),
                )

            # ================= stage 3: Linear2 (l1, l2, l0) =================
            for ns in range(NSUB):
                nsl = slice(ns * 128, (ns + 1) * 128)
                outsb = sb.tile([128, D_OUT], F32, name="outsb", tag="outsb", bufs=2)
                ov1 = outsb[:, 512:1280].rearrange("p (v j) -> p v j", j=3)
                ov2 = outsb[:, 1280:1920].rearrange("p (v j) -> p v j", j=5)
                # l1: out muls 256, contraction 512 -> ku 0..3
                q1 = ps.tile([128, 2, 256], F32, name="q1", tag="ps1", bufs=2)
                for idx, j in enumerate((0, 1)):
                    for ku in range(4):
                        nc.tensor.matmul(
                            q1[:, idx, :],
                            g_l1[:, ku, j, nsl],
                            w2r_l1[:, ku, :],
                            start=(ku == 0),
                            stop=(ku == 3),
                        )
                nc.scalar.activation(
                    out=ov1[:, :, 0:2], in_=q1.rearrange("p j v -> p v j"), func=AF.Copy
                )
                q2 = ps.tile([128, 2, 256], F32, name="q2", tag="ps1", bufs=2)
                for ku in range(4):
                    nc.tensor.matmul(
                        q2[:, 0, :],
                        g_l1[:, ku, 2, nsl],
                        w2r_l1[:, ku, :],
                        start=(ku == 0),
                        stop=(ku == 3),
                    )
                nc.scalar.activation(
                    out=ov1[:, :, 2:3],
                    in_=q2[:, 0:1, :].rearrange("p j v -> p v j"),
                    func=AF.Copy,
                )
                # l2: out muls 128, contraction 256 -> ku 0..1
                q3 = ps.tile([128, 4, 128], F32, name="q3", tag="ps1", bufs=2)
                for j in range(4):
                    for ku in range(2):
                        nc.tensor.matmul(
                            q3[:, j, :],
                            g_l2[:, ku, j, nsl],
                            w2r_l2[:, ku, :],
                            start=(ku == 0),
                            stop=(ku == 1),
                        )
                nc.scalar.activation(
                    out=ov2[:, :, 0:4], in_=q3.rearrange("p j v -> p v j"), func=AF.Copy
                )
                q4 = ps.tile([128, 4, 128], F32, name="q4", tag="ps1", bufs=2)
                for ku in range(2):
                    nc.tensor.matmul(
                        q4[:, 0, :],
                        g_l2[:, ku, 4, nsl],
                        w2r_l2[:, ku, :],
                        start=(ku == 0),
                        stop=(ku == 1),
                    )
                nc.scalar.activation(
                    out=ov2[:, :, 4:5],
                    in_=q4[:, 0:1, :].rearrange("p j v -> p v j"),
                    func=AF.Copy,
                )
                # l0: out muls 512, contraction 1024 -> ku 0..7
                q0 = ps.tile([128, 512], F32, name="q0", tag="ps1", bufs=2)
                for ku in range(8):
                    nc.tensor.matmul(
                        q0,
                        g_l0[:, ku, nsl],
                        w2r_l0[:, ku, :],
                        start=(ku == 0),
                        stop=(ku == 7),
                    )
                nc.scalar.activation(out=outsb[:, 0:512], in_=q0, func=AF.Copy)

                nc.sync.dma_start(
                    out=out[b * BLK + ns * 128 : b * BLK + (ns + 1) * 128, :],
                    in_=outsb,
                )

    nc.compile()
    return nc


_NC_CACHE = None


def _get_nc():
    global _NC_CACHE
    if _NC_CACHE is None:
        _NC_CACHE = _build()
    return _NC_CACHE


def kernel(x, w1_l0, w1_l1, w1_l2, w2_l0, w2_l1, w2_l2):
    x = np.ascontiguousarray(np.asarray(x, dtype=np.float32))
    ws = {
        "w1_l0": np.ascontiguousarray(
            np.asarray(w1_l0, np.float32) / math.sqrt(512.0)
        ),
        "w1_l1": np.ascontiguousarray(
            np.asarray(w1_l1, np.float32) / math.sqrt(256.0)
        ),
        "w1_l2": np.ascontiguousarray(
            np.asarray(w1_l2, np.float32) / math.sqrt(128.0)
        ),
        "w2_l0": np.ascontiguousarray(
            np.asarray(w2_l0, np.float32) / math.sqrt(1024.0)
        ),
        "w2_l1": np.ascontiguousarray(
            np.asarray(w2_l1, np.float32) / math.sqrt(512.0)
        ),
        "w2_l2": np.ascontiguousarray(
            np.asarray(w2_l2, np.float32) / math.sqrt(256.0)
        ),
    }
    nc = _get_nc()
    in_maps = [
        {"x": x[c * N_CORE : (c + 1) * N_CORE], **ws} for c in range(N_CORES)
    ]
    res = run_bass_kernel_spmd(nc, in_maps, list(range(N_CORES))).results
    return np.concatenate([res[c]["out"] for c in range(N_CORES)], axis=0)
